# revision 29
# baseline (speedup 1.0000x reference)
"""CLUB loss kernel for 8 trn2 NeuronCores — fp8 DoubleRow edition.

Math (reference):
    mu     = relu(z_c @ W1m + b1m) @ W2m + b2m
    logvar = tanh(relu(z_c @ W1l + b1l) @ W2l + b2l)
    iv'    = exp(-logvar)
    N*mi   = (s1 - s2/2) + sum_d Q_d*A_d/(2N) - sum_d P_d*B_d/N
      A = sum_i iv'        B = sum_i iv'*mu      (per-d, on device)
      s1 = sum iv'*mu*zd   s2 = sum iv'*zd^2     (scalars, on device)
      P = sum_i zd         Q = sum_i zd^2        (host, f64, from the same
                                                  fp16 zd/zd^2 the device uses)

Sharding: data-parallel over N (2048 rows/core), weights replicated; host
combines the tiny O(D) partials in f64 (no device collective).

Precision plan (keeps rel err ~2e-3, gate is 2e-2):
  mu-L1  fp16          (loss is exactly linear in mu, but L1 noise is costly)
  mu-L2  fp8 DoubleRow (0.5 cyc/row)
  lv-L1  fp8 DoubleRow
  lv-L2  fp8 DoubleRow
z_c / z_d arrive HOST-TRANSPOSED (feature-major), so the PE does zero
transposes. fp8 host quantization flushes e4m3 subnormals (measured better).

DoubleRow constraint: matmul dst must start at PSUM partition 0 with <=64
out-partitions. Odd 64-feature halves are produced on partitions 0..63 and
relocated to partitions 64..127 by SBUF->SBUF DMAs (h planes, tanh input,
evacuated mu), after which everything downstream runs full-width [128, 512].
"""

import sys

if "/opt/trn_rl_repo" not in sys.path:
    sys.path.insert(0, "/opt/trn_rl_repo")

import numpy as np
import ml_dtypes

import concourse.bacc as bacc
import concourse.mybir as mybir
import concourse.tile as tile
from concourse.bass_utils import run_bass_kernel_spmd

N, DC, H, DD = 16384, 1024, 1024, 1024
NCORES = 8
R = N // NCORES          # rows per core
F = 512                  # rows per block (PSUM bank = 512 f32)
NB = R // F
KC = DC // 128           # 128-blocks in contractions
KP = DC // 256           # 256-pairs for DoubleRow
CC = DD // 128           # 128-blocks of DD
NIDX = NB * 8            # accumulator columns per quantity (b*8+c)

F32 = mybir.dt.float32
F16 = mybir.dt.float16
F8 = mybir.dt.float8e4
AF = mybir.ActivationFunctionType
OP = mybir.AluOpType
DR = mybir.MatmulPerfMode.DoubleRow
E4 = ml_dtypes.float8_e4m3

# precision flags (config "c"); flip LV2/MU2 to False for the safe config "a"
LV1_FP8 = True
LV2_FP8 = True
MU2_FP8 = True
FTZ = True               # flush e4m3 subnormals in host quantization

_CACHE = {}

# bias column layout in the [128, 64] biases tensor
BC_B1M = 0    # [128, 8]  natural: col m <-> features m*128+p
BC_B2M = 8    # [128, 8]  natural (fp16 path) / even halves [0:64]
BC_B2MO = 16  # [64, 8]   b2m odd halves: [j, c] = b2m[128c+64+j]
BC_B1L = 24   # [64, 16]  m64 layout: [j, a] = b1l[64a+j]
BC_B2L = 40   # [128, 8]  natural; [0:64, c] doubles as even halves
BC_B2LO = 48  # [64, 8]   b2l odd halves
BC_B1L128 = 56  # [128, 8] b1l natural (fp16 lv-L1 fallback)


def _build():
    nc = bacc.Bacc("TRN2", num_devices=NCORES)

    zc16t = nc.declare_dram_parameter("zc16t", [128, KC, R], F16, isOutput=False)
    zc8t = nc.declare_dram_parameter("zc8t", [128, KC, R], F8, isOutput=False)
    zdt_in = nc.declare_dram_parameter("zdt", [128, CC, R], F16, isOutput=False)
    zd2t_in = nc.declare_dram_parameter("zd2t", [128, CC, R], F16, isOutput=False)
    w1m_in = nc.declare_dram_parameter("w1m", [1024, 1024], F16, isOutput=False)
    if MU2_FP8:
        w2m_in = nc.declare_dram_parameter("w2m", [128, KP, 16, 2, 64], F8, isOutput=False)
    else:
        w2m_in = nc.declare_dram_parameter("w2m", [1024, 1024], F16, isOutput=False)
    w1l_in = nc.declare_dram_parameter("w1l", [128, KP, 16, 2, 64], F8, isOutput=False)
    if LV2_FP8:
        w2l_in = nc.declare_dram_parameter("w2l", [128, KP, 16, 2, 64], F8, isOutput=False)
    else:
        w2l_in = nc.declare_dram_parameter("w2l", [128, KC, 1024], F16, isOutput=False)
    bias_in = nc.declare_dram_parameter("biases", [128, 64], F32, isOutput=False)
    out_a = nc.declare_dram_parameter("acc_a", [128, NIDX], F32, isOutput=True)
    out_d = nc.declare_dram_parameter("acc_d", [128, 3 * NIDX], F32, isOutput=True)

    from contextlib import ExitStack

    with tile.TileContext(nc) as tc, ExitStack() as es:
        wpool = es.enter_context(tc.tile_pool(name="wpool", bufs=1))
        st8 = es.enter_context(tc.tile_pool(name="st8", bufs=2))
        st16 = es.enter_context(tc.tile_pool(name="st16", bufs=2))
        stz = es.enter_context(tc.tile_pool(name="stz", bufs=2))
        stz2 = es.enter_context(tc.tile_pool(name="stz2", bufs=2))
        h_p = es.enter_context(tc.tile_pool(name="hp", bufs=2))
        ls_p = es.enter_context(tc.tile_pool(name="ls", bufs=3))
        acc_p = es.enter_context(tc.tile_pool(name="accp", bufs=1))
        mu1_ps = es.enter_context(tc.tile_pool(name="mu1ps", bufs=2, space="PSUM"))
        # all DoubleRow [64,512] psums share one 6-deep rotation (6 banks)
        dr_ps = es.enter_context(tc.tile_pool(name="drps", bufs=6, space="PSUM"))
        lv1_ps = lv2_ps = mu2_ps = dr_ps

        # ---- startup DMAs, critical-path order ----
        def stage(pool, src, b, dt, nm):
            t = pool.tile([128, src.shape[1], F], dt, tag=f"st_{nm}", name=f"st_{nm}_{b}")
            nc.sync.dma_start(t[:], src[:, :, b * F:(b + 1) * F])
            return t

        zc8_st0 = stage(st8, zc8t, 0, F8, "zc8")
        w1l = wpool.tile([128, KP, 16, 2, 64], F8, tag="w1l")
        nc.sync.dma_start(w1l[:], w1l_in[:])
        ball = acc_p.tile([128, 64], F32, tag="ball")
        nc.sync.dma_start(ball[:], bias_in[:])
        zc16_st0 = stage(st16, zc16t, 0, F16, "zc16")
        w1m = {}
        for k in range(KC):
            t = wpool.tile([128, 1024], F16, tag=f"w1m{k}")
            nc.sync.dma_start(t[:], w1m_in[k * 128:(k + 1) * 128, :])
            w1m[k] = t
        if LV2_FP8:
            w2l = wpool.tile([128, KP, 16, 2, 64], F8, tag="w2l")
            nc.sync.dma_start(w2l[:], w2l_in[:])
        else:
            w2l = {}
            for k in range(KC):
                t = wpool.tile([128, 1024], F16, tag=f"w2l{k}")
                nc.sync.dma_start(t[:], w2l_in[:, k, :])
                w2l[k] = t
        if MU2_FP8:
            w2m = wpool.tile([128, KP, 16, 2, 64], F8, tag="w2m")
            nc.sync.dma_start(w2m[:], w2m_in[:])
        else:
            w2m = {}
            for k in range(KC):
                t = wpool.tile([128, 1024], F16, tag=f"w2m{k}")
                nc.sync.dma_start(t[:], w2m_in[k * 128:(k + 1) * 128, :])
                w2m[k] = t
        zdt_st0 = stage(stz, zdt_in, 0, F16, "zd")
        zd2_st0 = stage(stz2, zd2t_in, 0, F16, "zd2")

        acc_a = acc_p.tile([128, NIDX], F32, tag="acc_a")
        acc_d = acc_p.tile([128, 3 * NIDX], F32, tag="acc_d")

        hlv_dt = F8 if LV2_FP8 else F16
        hmu_dt = F8 if MU2_FP8 else F16

        for b in range(NB):
            if b == 0:
                zc8_st, zc16_st, zdt_st, zd2_st = zc8_st0, zc16_st0, zdt_st0, zd2_st0
            else:
                zc8_st = stage(st8, zc8t, b, F8, "zc8")
                zc16_st = stage(st16, zc16t, b, F16, "zc16")
                zdt_st = stage(stz, zdt_in, b, F16, "zd")
                zd2_st = stage(stz2, zd2t_in, b, F16, "zd2")

            # ---- L1: lv (fp8 DoubleRow) interleaved with mu (fp16) ----
            # Interleaving keeps the PE busy on mu matmuls while the Act
            # engine drains the 16 lv relu+quantize ops (relu is slower than
            # a DoubleRow psum, so lv alone would stall the psum rotation).
            h_lv = h_p.tile([128, KC, F], hlv_dt, tag="h_lv", name=f"h_lv_{b}")
            hstage = h_p.tile([64, KC, F], hlv_dt, tag="hstage", name=f"hstage_{b}")
            h_mu = h_p.tile([128, KC, F], hmu_dt, tag="h_mu", name=f"h_mu_{b}")

            def lv1(a):
                ps = lv1_ps.tile([64, F], F32, tag="dr", name=f"lv1ps_{b}_{a}")
                for g in range(2):
                    for kp in range(KP):
                        nc.tensor.matmul(
                            ps[:, g * 256:(g + 1) * 256],
                            w1l[:, kp, a, :, :],
                            zc8_st[:, 2 * kp:2 * kp + 2, g * 256:(g + 1) * 256],
                            start=(kp == 0), stop=(kp == KP - 1), perf_mode=DR,
                        )
                hs, par = a // 2, a % 2
                dst = h_lv[0:64, hs, :] if par == 0 else hstage[:, hs, :]
                nc.scalar.activation(
                    dst, ps[:], AF.Relu,
                    bias=ball[0:64, BC_B1L + a:BC_B1L + a + 1], scale=1.0 / 256.0,
                )

            def mu1(m):
                ps = mu1_ps.tile([128, F], F32, tag="mu1", name=f"mu1ps_{b}_{m}")
                for k in range(KC):
                    nc.tensor.matmul(
                        ps[:], w1m[k][:, m * 128:(m + 1) * 128], zc16_st[:, k, :],
                        start=(k == 0), stop=(k == KC - 1),
                    )
                # relu on DVE: (ps + b1m) max 0 -> fp8/f16
                nc.vector.tensor_scalar(
                    h_mu[:, m, :], ps[:], ball[:, BC_B1M + m:BC_B1M + m + 1], 0.0,
                    OP.add, OP.max,
                )

            for i in range(KC):
                lv1(2 * i)
                lv1(2 * i + 1)
                mu1(i)
            nc.sync.dma_start(h_lv[64:128, :, :], hstage[:, :, :])

            # ---- per output-feature chunk c ----
            for c in range(8):
                i1 = b * 8 + c

                # lv-L2 -> lg (tanh), iv (exp, accum A)
                lg = ls_p.tile([128, F], F16, tag="lg")
                if LV2_FP8:
                    ps_e = lv2_ps.tile([64, F], F32, tag="dr")
                    ps_o = lv2_ps.tile([64, F], F32, tag="dr")
                    for half, pst in ((0, ps_e), (1, ps_o)):
                        d64 = 2 * c + half
                        for g in range(2):
                            for kp in range(KP):
                                nc.tensor.matmul(
                                    pst[:, g * 256:(g + 1) * 256],
                                    w2l[:, kp, d64, :, :],
                                    h_lv[:, 2 * kp:2 * kp + 2, g * 256:(g + 1) * 256],
                                    start=(kp == 0), stop=(kp == KP - 1), perf_mode=DR,
                                )
                    nc.scalar.activation(
                        lg[0:64, :], ps_e[:], AF.Tanh,
                        bias=ball[0:64, BC_B2L + c:BC_B2L + c + 1], scale=1.0 / 256.0,
                    )
                    lgst = ls_p.tile([64, F], F16, tag="lgst")
                    nc.scalar.activation(
                        lgst[:], ps_o[:], AF.Tanh,
                        bias=ball[0:64, BC_B2LO + c:BC_B2LO + c + 1], scale=1.0 / 256.0,
                    )
                    nc.sync.dma_start(lg[64:128, :], lgst[:])
                else:
                    ps_lv = lv2_ps.tile([128, F], F32, tag="lv2")
                    for m in range(KC):
                        nc.tensor.matmul(
                            ps_lv[:], w2l[m][:, c * 128:(c + 1) * 128], h_lv[:, m, :],
                            start=(m == 0), stop=(m == KC - 1),
                        )
                    nc.scalar.activation(
                        lg[:], ps_lv[:], AF.Tanh,
                        bias=ball[:, BC_B2L + c:BC_B2L + c + 1],
                    )
                iv = ls_p.tile([128, F], F16, tag="iv")
                nc.scalar.activation(
                    iv[:], lg[:], AF.Exp, scale=-1.0,
                    accum_out=acc_a[:, i1:i1 + 1],
                )

                # mu-L2 -> mu (+b2m), then g = mu*iv (accum B)
                g_t = ls_p.tile([128, F], F16, tag="g")
                if MU2_FP8:
                    ps_me = mu2_ps.tile([64, F], F32, tag="dr")
                    ps_mo = mu2_ps.tile([64, F], F32, tag="dr")
                    for half, pst in ((0, ps_me), (1, ps_mo)):
                        d64 = 2 * c + half
                        for g in range(2):
                            for kp in range(KP):
                                nc.tensor.matmul(
                                    pst[:, g * 256:(g + 1) * 256],
                                    w2m[:, kp, d64, :, :],
                                    h_mu[:, 2 * kp:2 * kp + 2, g * 256:(g + 1) * 256],
                                    start=(kp == 0), stop=(kp == KP - 1), perf_mode=DR,
                                )
                    must = ls_p.tile([128, F], F16, tag="must")
                    # mu evacuation: DVE in steady state; Act (Identity
                    # with bias) for the final chunks where Act is idle and
                    # DVE is the drain-chain bottleneck.
                    mustg = ls_p.tile([64, F], F16, tag="mustg")
                    if b == NB - 1 and c >= 7:
                        nc.scalar.activation(
                            must[0:64, :], ps_me[:], AF.Identity,
                            bias=ball[0:64, BC_B2M + c:BC_B2M + c + 1],
                            scale=1.0 / 256.0,
                        )
                        nc.scalar.activation(
                            mustg[:], ps_mo[:], AF.Identity,
                            bias=ball[0:64, BC_B2MO + c:BC_B2MO + c + 1],
                            scale=1.0 / 256.0,
                        )
                    else:
                        nc.vector.tensor_scalar(
                            must[0:64, :], ps_me[:], 1.0 / 256.0,
                            ball[0:64, BC_B2M + c:BC_B2M + c + 1], OP.mult, OP.add,
                        )
                        nc.vector.tensor_scalar(
                            mustg[:], ps_mo[:], 1.0 / 256.0,
                            ball[0:64, BC_B2MO + c:BC_B2MO + c + 1], OP.mult, OP.add,
                        )
                    nc.sync.dma_start(must[64:128, :], mustg[:])
                    nc.vector.scalar_tensor_tensor(
                        g_t[:], must[:], 0.0, iv[:], op0=OP.add, op1=OP.mult,
                        accum_out=acc_d[:, i1:i1 + 1],
                    )
                else:
                    ps_mu = mu2_ps.tile([128, F], F32, tag="mu2")
                    for m in range(KC):
                        nc.tensor.matmul(
                            ps_mu[:], w2m[m][:, c * 128:(c + 1) * 128], h_mu[:, m, :],
                            start=(m == 0), stop=(m == KC - 1),
                        )
                    nc.vector.scalar_tensor_tensor(
                        g_t[:], ps_mu[:], ball[:, BC_B2M + c:BC_B2M + c + 1], iv[:],
                        op0=OP.add, op1=OP.mult,
                        accum_out=acc_d[:, i1:i1 + 1],
                    )

                # s1 += sum(g*zd); s2 += sum(iv*zd2).  Pool does the
                # products (only plain tensor_tensor runs there); DVE does the
                # cheap 4x-mode accumulating copies.  In the last block the
                # slow Pool ops sit on the end-of-kernel drain chain, so the
                # products run as DVE 2x tensor_tensor instead.
                prod_eng = nc.vector if b == NB - 1 else nc.gpsimd
                scr = ls_p.tile([128, F], F16, tag="scr")
                prod_eng.tensor_tensor(scr[:], g_t[:], zdt_st[:, c, :], OP.mult)
                scr1a = ls_p.tile([128, F], F16, tag="scr1a")
                nc.vector.tensor_scalar(
                    scr1a[:], scr[:], 0.0, 0.0, OP.add, OP.add,
                    accum_out=acc_d[:, NIDX + i1:NIDX + i1 + 1],
                )
                scr2 = ls_p.tile([128, F], F16, tag="scr2")
                prod_eng.tensor_tensor(scr2[:], iv[:], zd2_st[:, c, :], OP.mult)
                scr2a = ls_p.tile([128, F], F16, tag="scr2a")
                nc.vector.tensor_scalar(
                    scr2a[:], scr2[:], 0.0, 0.0, OP.add, OP.add,
                    accum_out=acc_d[:, 2 * NIDX + i1:2 * NIDX + i1 + 1],
                )

        nc.sync.dma_start(out_a[:], acc_a[:])
        nc.sync.dma_start(out_d[:], acc_d[:])

    nc.compile()
    return nc


def _q8(x, scale=1.0):
    v = np.asarray(x * scale, dtype=E4)
    if FTZ:
        f = v.astype(np.float32)
        v[np.abs(f) < 2.0 ** -6] = 0
    return v


def _dr_weights(W, fmap=None):
    """[1024,1024] -> DoubleRow stationary [128, KP, 16, 2, 64] (fp8, x256)."""
    W8 = _q8(W, 256.0)
    out = np.zeros((128, KP, 16, 2, 64), dtype=E4)
    p = np.arange(128)
    for kp in range(KP):
        for i in range(2):
            ks = 2 * kp + i
            rows = fmap(p, ks) if fmap is not None else ks * 128 + p
            out[:, kp, :, i, :] = W8[rows].reshape(128, 16, 64)
    return out


def kernel(z_c, z_d, W1_mu, b1_mu, W2_mu, b2_mu, W1_lv, b1_lv, W2_lv, b2_lv):
    if "nc" not in _CACHE:
        _CACHE["nc"] = _build()
    nc = _CACHE["nc"]

    fmap = lambda p, hs: 64 * (2 * hs + p // 64) + (p % 64)  # h_lv parity map

    biases = np.zeros((128, 64), dtype=np.float32)
    biases[:, BC_B1M:BC_B1M + 8] = b1_mu.reshape(8, 128).T
    biases[:, BC_B2M:BC_B2M + 8] = b2_mu.reshape(8, 128).T
    biases[0:64, BC_B2MO:BC_B2MO + 8] = b2_mu.reshape(8, 2, 64)[:, 1, :].T
    biases[0:64, BC_B1L:BC_B1L + 16] = b1_lv.reshape(16, 64).T
    biases[:, BC_B2L:BC_B2L + 8] = b2_lv.reshape(8, 128).T
    biases[0:64, BC_B2LO:BC_B2LO + 8] = b2_lv.reshape(8, 2, 64)[:, 1, :].T
    biases[:, BC_B1L128:BC_B1L128 + 8] = b1_lv.reshape(8, 128).T

    common = {
        "w1m": np.ascontiguousarray(W1_mu.astype(np.float16)),
        "w1l": _dr_weights(W1_lv),
        "biases": biases,
    }
    if MU2_FP8:
        common["w2m"] = _dr_weights(W2_mu)
    else:
        common["w2m"] = np.ascontiguousarray(W2_mu.astype(np.float16))
    if LV2_FP8:
        common["w2l"] = _dr_weights(W2_lv, fmap=fmap)
    else:
        w2l16 = W2_lv.astype(np.float16)
        wl = np.zeros((128, KC, 1024), dtype=np.float16)
        p = np.arange(128)
        for hs in range(KC):
            wl[:, hs, :] = w2l16[fmap(p, hs)]
        common["w2l"] = np.ascontiguousarray(wl)

    zc16 = np.asarray(z_c, dtype=np.float16)
    zc8 = _q8(np.asarray(z_c, dtype=np.float32))
    zd16 = np.asarray(z_d, dtype=np.float16)
    zd2_16 = (zd16.astype(np.float32) ** 2).astype(np.float16)

    def tview(x, nblk):  # [R, 1024] -> [128, nblk, R] feature-major
        return np.ascontiguousarray(
            x.T.reshape(nblk, 128, x.shape[0]).transpose(1, 0, 2))

    in_maps = []
    for i in range(NCORES):
        sl = slice(i * R, (i + 1) * R)
        in_maps.append({
            "zc16t": tview(zc16[sl], KC),
            "zc8t": tview(zc8[sl], KC),
            "zdt": tview(zd16[sl], CC),
            "zd2t": tview(zd2_16[sl], CC),
            **common,
        })

    res = run_bass_kernel_spmd(nc, in_maps, list(range(NCORES)))

    def vec(cols):  # [128, NIDX] -> [DD] summed over blocks; d = c*128 + p
        v = cols.astype(np.float64).reshape(128, NB, 8).sum(axis=1)
        return v.T.reshape(DD)

    A = np.zeros(DD)
    B = np.zeros(DD)
    s1 = 0.0
    s2 = 0.0
    for i in range(NCORES):
        oa = res.results[i]["acc_a"]
        od = res.results[i]["acc_d"]
        A += vec(oa)
        B += vec(od[:, :NIDX])
        s1 += od[:, NIDX:2 * NIDX].astype(np.float64).sum()
        s2 += od[:, 2 * NIDX:].astype(np.float64).sum()

    P = zd16.astype(np.float64).sum(axis=0)
    Q = zd2_16.astype(np.float64).sum(axis=0)

    total = (s1 - 0.5 * s2) + float(Q @ A) / (2.0 * N) - float(P @ B) / N
    return np.asarray(total / N, dtype=np.float32)
